# revision 1
# baseline (speedup 1.0000x reference)
"""Trainium2 Bass kernel for nn_Decoder (GRU decoder over phenotype codes).

Computation (see reference):
  h0 = W_cw @ phe_flat + b_cw                       (256,)
  G  = H.T @ X                                      (100, 256)   big GEMM, sharded over codes
  emb[t] = relu(G[t-1]), emb[0] = 0
  gates_x = emb @ W_ih.T + b_ih                     (100, 768)
  GRU scan over 100 steps (hidden 256)
  preds = sigmoid(hs @ W_out.T + W_out_b).T         (50000, 100)

Sharding: codes (50000) split over 8 cores (6250 -> padded 6272 each).
Front GEMM is a partial sum per core -> AllReduce(100x256) -> everything
after the AllReduce up to the output GEMM is replicated; output GEMM and
sigmoid are sharded over codes again.
"""

import os
import sys

import numpy as np

sys.path.insert(0, "/opt/trn_rl_repo")

import concourse.bass as bass  # noqa: E402
import concourse.mybir as mybir  # noqa: E402
import concourse.tile as tile  # noqa: E402
from concourse import bacc, bass_utils  # noqa: E402
from concourse.masks import make_identity  # noqa: E402

HID = 256
NTP = 8
EMB = 256
NC = 50000
NV = 100
NCORES = 8
SH = NC // NCORES  # 6250
KT = (SH + 127) // 128  # 49 k-tiles per shard
SHP = KT * 128  # 6272 padded codes per core

F32 = mybir.dt.float32
BF16 = mybir.dt.bfloat16

# dtype knobs (flip to BF16 for bandwidth/weight-load savings)
DT_FRONT = F32  # H, X and the G GEMM
DT_SCAN = F32  # W_hh, h state, GX, identity
DT_OUT = F32  # W_out and the output GEMM

_NP = {F32: np.float32, BF16: None}


def _npdt(dt):
    if dt == F32:
        return np.float32
    import ml_dtypes

    return ml_dtypes.bfloat16


def build_program():
    nc = bacc.Bacc(
        "TRN2",
        target_bir_lowering=False,
        debug=False,
        enable_asserts=False,
        num_devices=NCORES,
    )

    # ---- DRAM I/O (per core) ----
    H_d = nc.dram_tensor("h_mat", [SHP, NV], DT_FRONT, kind="ExternalInput")
    X_d = nc.dram_tensor("x_mat", [SHP, EMB], DT_FRONT, kind="ExternalInput")
    WoT_d = nc.dram_tensor("wout_t", [EMB, SHP], DT_OUT, kind="ExternalInput")
    Wob_d = nc.dram_tensor("wout_b", [128, KT], F32, kind="ExternalInput")
    phe_d = nc.dram_tensor("phe", [128, 16], F32, kind="ExternalInput")
    WcwT_d = nc.dram_tensor("wcw_t", [NTP * HID, HID], F32, kind="ExternalInput")
    bcw_d = nc.dram_tensor("wcw_b", [128, 2], F32, kind="ExternalInput")
    WihT_d = nc.dram_tensor("wih_t", [EMB, 3 * HID], DT_SCAN, kind="ExternalInput")
    bih_d = nc.dram_tensor("gates_bias", [1, 3 * HID], DT_SCAN, kind="ExternalInput")
    WhhT_d = nc.dram_tensor("whh_t", [HID, 3 * HID], DT_SCAN, kind="ExternalInput")
    bhn_d = nc.dram_tensor("bhn", [128, 2], DT_SCAN, kind="ExternalInput")
    out_d = nc.dram_tensor("preds", [SHP, NV], F32, kind="ExternalOutput")

    with tile.TileContext(nc) as tc:
        with (
            tc.tile_pool(name="const", bufs=1) as cpool,
            tc.tile_pool(name="big", bufs=1) as bigpool,
            tc.tile_pool(name="work", bufs=2) as wpool,
            tc.tile_pool(name="dram", bufs=1, space="DRAM") as dpool,
        ):
            # ---- persistent SBUF tiles ----
            phe_sb = cpool.tile([128, 16], F32)
            WcwT_sb = cpool.tile([128, 16, HID], F32)
            bcw_sb = cpool.tile([128, 2], F32)
            WihT_sb = cpool.tile([128, 2, 3 * HID], DT_SCAN)
            bih_sb = cpool.tile([1, 3 * HID], DT_SCAN)
            WhhT_sb = cpool.tile([128, 2, 3 * HID], DT_SCAN)
            bhn_sb = cpool.tile([128, 2], DT_SCAN)
            Wob_sb = cpool.tile([128, KT], F32)
            ID_sb = cpool.tile([128, 128], DT_SCAN)
            ones_sb = cpool.tile([1, NV], DT_SCAN)
            H_sb = bigpool.tile([128, KT, NV], DT_FRONT)
            X_sb = bigpool.tile([128, KT, EMB], DT_FRONT)
            WoT_sb = bigpool.tile([128, 2, SHP], DT_OUT)
            G_sb = cpool.tile([NV, EMB], F32)
            Gr_sb = cpool.tile([NV, EMB], DT_SCAN)
            embT_sb = cpool.tile([128, 2, NV], DT_SCAN)
            GX_sb = cpool.tile([128, NV, 4], DT_SCAN)
            GXN_sb = cpool.tile([128, NV, 2], DT_SCAN)
            hs_sb = cpool.tile([128, 2, NV], DT_SCAN)
            h0_sb = cpool.tile([128, 2], DT_SCAN)

            ar_in = dpool.tile([NV, EMB], F32)
            ar_out = dpool.tile([NV, EMB], F32)

            # ---- small DMAs + constants ----
            nc.sync.dma_start(phe_sb[:], phe_d[:])
            nc.sync.dma_start(
                WcwT_sb[:], WcwT_d.ap().rearrange("(k p) e -> p k e", p=128)
            )
            nc.sync.dma_start(bcw_sb[:], bcw_d[:])
            nc.sync.dma_start(
                WihT_sb[:], WihT_d.ap().rearrange("(c p) m -> p c m", p=128)
            )
            nc.sync.dma_start(bih_sb[:], bih_d[:])
            nc.sync.dma_start(
                WhhT_sb[:], WhhT_d.ap().rearrange("(c p) m -> p c m", p=128)
            )
            nc.sync.dma_start(bhn_sb[:], bhn_d[:])
            nc.sync.dma_start(Wob_sb[:], Wob_d[:])
            make_identity(nc, ID_sb[:])
            nc.gpsimd.memset(ones_sb[:], 1.0)
            nc.gpsimd.memset(embT_sb[:], 0.0)

            # ---- big DMAs ----
            nc.sync.dma_start(H_sb[:], H_d.ap().rearrange("(k p) v -> p k v", p=128))
            nc.sync.dma_start(X_sb[:], X_d.ap().rearrange("(k p) e -> p k e", p=128))
            nc.sync.dma_start(WoT_sb[:], WoT_d.ap().rearrange("(c p) n -> p c n", p=128))

            # ---- h0 = W_cw @ phe + b_cw ----
            with tc.tile_pool(name="ps_h0", bufs=1, space="PSUM") as pp:
                ps_h0 = pp.tile([128, 2], F32)
                for m in range(2):
                    for k in range(16):
                        nc.tensor.matmul(
                            ps_h0[:, m : m + 1],
                            WcwT_sb[:, k, m * 128 : (m + 1) * 128],
                            phe_sb[:, k : k + 1],
                            start=(k == 0),
                            stop=(k == 15),
                        )
                nc.vector.tensor_add(h0_sb[:], ps_h0[:], bcw_sb[:])

            # ---- front GEMM: G_partial = H_c.T @ X_c  (100 x 256) ----
            with tc.tile_pool(name="ps_g", bufs=1, space="PSUM") as pp:
                ps_g = pp.tile([NV, EMB], F32)
                for k in range(KT):
                    nc.tensor.matmul(
                        ps_g[:],
                        H_sb[:, k, :],
                        X_sb[:, k, :],
                        start=(k == 0),
                        stop=(k == KT - 1),
                    )
                Gp_sb = wpool.tile([NV, EMB], F32)
                nc.vector.tensor_copy(Gp_sb[:], ps_g[:])

            # ---- AllReduce the partial G ----
            nc.sync.dma_start(ar_in[:], Gp_sb[:])
            nc.gpsimd.collective_compute(
                "AllReduce",
                mybir.AluOpType.add,
                replica_groups=[list(range(NCORES))],
                ins=[ar_in.opt()],
                outs=[ar_out.opt()],
            )
            nc.sync.dma_start(G_sb[:], ar_out[:])

            # ---- emb.T = shifted relu(G).T  (256 x 100, col 0 = zeros) ----
            nc.vector.tensor_scalar(Gr_sb[:], G_sb[:], 0.0, None, mybir.AluOpType.max)
            with tc.tile_pool(name="ps_tr", bufs=2, space="PSUM") as pp:
                for c in range(2):
                    ps_tr = pp.tile([128, NV - 1], F32, tag="tr")
                    nc.tensor.transpose(
                        ps_tr[:],
                        Gr_sb[0 : NV - 1, c * 128 : (c + 1) * 128],
                        ID_sb[0 : NV - 1, 0 : NV - 1],
                    )
                    nc.vector.tensor_copy(embT_sb[:, c, 1:NV], ps_tr[:])

            # ---- gates_x GEMM -> GX (rz, interleaved) and GXN (n) ----
            with tc.tile_pool(name="ps_gx", bufs=3, space="PSUM") as pp:
                for m in range(6):
                    ps_gx = pp.tile([128, NV], F32, tag="gx")
                    for kc in range(2):
                        nc.tensor.matmul(
                            ps_gx[:],
                            WihT_sb[:, kc, m * 128 : (m + 1) * 128],
                            embT_sb[:, kc, :],
                            start=(kc == 0),
                            stop=False,
                        )
                    nc.tensor.matmul(
                        ps_gx[:],
                        bih_sb[0:1, m * 128 : (m + 1) * 128],
                        ones_sb[0:1, :],
                        start=False,
                        stop=True,
                    )
                    if m < 4:
                        nc.vector.tensor_copy(GX_sb[:, :, m], ps_gx[:])
                    else:
                        nc.vector.tensor_copy(GXN_sb[:, :, m - 4], ps_gx[:])

            # ---- GRU scan ----
            with (
                tc.tile_pool(name="ps_scan", bufs=2, space="PSUM") as pp,
                tc.tile_pool(name="scan_t", bufs=3) as sp,
            ):
                for t in range(NV):
                    hcur = h0_sb if t == 0 else hs_sb[:, :, t - 1]
                    ps = pp.tile([128, 6], F32, tag="scan")
                    # seed psum with gate biases / rz inputs (sets has_written)
                    nc.tensor.matmul(
                        ps[:, 0:4], ID_sb[:], GX_sb[:, t, :], start=True, stop=False
                    )
                    nc.tensor.matmul(
                        ps[:, 4:6], ID_sb[:], bhn_sb[:], start=True, stop=False
                    )
                    for m in range(6):
                        for kc in range(2):
                            nc.tensor.matmul(
                                ps[:, m : m + 1],
                                WhhT_sb[:, kc, m * 128 : (m + 1) * 128],
                                hcur[:, kc : kc + 1],
                                start=False,
                                stop=(m == 5 and kc == 1),
                            )
                    rz = sp.tile([128, 4], DT_SCAN, tag="rz")
                    nc.scalar.activation(
                        rz[:], ps[:, 0:4], mybir.ActivationFunctionType.Sigmoid
                    )
                    t1 = sp.tile([128, 2], DT_SCAN, tag="t1")
                    nc.vector.tensor_mul(t1[:], rz[:, 0:2], ps[:, 4:6])
                    t2 = sp.tile([128, 2], DT_SCAN, tag="t2")
                    nc.vector.tensor_add(t2[:], t1[:], GXN_sb[:, t, :])
                    n_t = sp.tile([128, 2], DT_SCAN, tag="n")
                    nc.scalar.activation(
                        n_t[:], t2[:], mybir.ActivationFunctionType.Tanh
                    )
                    w_t = sp.tile([128, 2], DT_SCAN, tag="w")
                    nc.vector.tensor_scalar(
                        w_t[:],
                        rz[:, 2:4],
                        -1.0,
                        1.0,
                        mybir.AluOpType.mult,
                        mybir.AluOpType.add,
                    )
                    u_t = sp.tile([128, 2], DT_SCAN, tag="u")
                    nc.vector.tensor_mul(u_t[:], rz[:, 2:4], hcur)
                    v_t = sp.tile([128, 2], DT_SCAN, tag="v")
                    nc.vector.tensor_mul(v_t[:], n_t[:], w_t[:])
                    nc.vector.tensor_add(hs_sb[:, :, t], v_t[:], u_t[:])

            # ---- output GEMM + sigmoid + store ----
            with (
                tc.tile_pool(name="ps_out", bufs=4, space="PSUM") as pp,
                tc.tile_pool(name="pred", bufs=4) as op,
            ):
                for m in range(KT):
                    ps_o = pp.tile([128, NV], F32, tag="o")
                    for kc in range(2):
                        nc.tensor.matmul(
                            ps_o[:],
                            WoT_sb[:, kc, m * 128 : (m + 1) * 128],
                            hs_sb[:, kc, :],
                            start=(kc == 0),
                            stop=(kc == 1),
                        )
                    pred = op.tile([128, NV], F32, tag="p")
                    nc.scalar.activation(
                        pred[:],
                        ps_o[:],
                        mybir.ActivationFunctionType.Sigmoid,
                        bias=Wob_sb[:, m : m + 1],
                    )
                    nc.sync.dma_start(out_d[m * 128 : (m + 1) * 128, :], pred[:])

    nc.compile()
    return nc


_PROG = None


def _get_program():
    global _PROG
    if _PROG is None:
        _PROG = build_program()
    return _PROG


def _pack2(v):
    # (256,) -> (128, 2) chunk-major: col c = v[128c:128c+128]
    return np.ascontiguousarray(v.reshape(2, 128).T)


def kernel(
    phenotype_embs,
    H,
    X,
    W_context_w,
    W_context_b,
    gru_W_ih,
    gru_W_hh,
    gru_b_ih,
    gru_b_hh,
    W_out_w,
    W_out_b,
    num_visits,
):
    assert int(num_visits) == NV
    fdt = _npdt(DT_FRONT)
    sdt = _npdt(DT_SCAN)
    odt = _npdt(DT_OUT)

    H = np.asarray(H, np.float32)
    X = np.asarray(X, np.float32)
    W_out_w = np.asarray(W_out_w, np.float32)
    W_out_b = np.asarray(W_out_b, np.float32)

    # replicated small tensors
    phe = np.ascontiguousarray(
        np.asarray(phenotype_embs, np.float32).reshape(-1).reshape(16, 128).T
    )
    WcwT = np.ascontiguousarray(np.asarray(W_context_w, np.float32).T)
    bcw = _pack2(np.asarray(W_context_b, np.float32))
    WihT = np.ascontiguousarray(np.asarray(gru_W_ih, np.float32).T).astype(sdt)
    b_ih = np.asarray(gru_b_ih, np.float32)
    b_hh = np.asarray(gru_b_hh, np.float32)
    gates_bias = b_ih.copy()
    gates_bias[: 2 * HID] += b_hh[: 2 * HID]
    gates_bias = gates_bias.reshape(1, -1).astype(sdt)
    WhhT = np.ascontiguousarray(np.asarray(gru_W_hh, np.float32).T).astype(sdt)
    bhn = _pack2(b_hh[2 * HID :]).astype(sdt)

    in_maps = []
    for c in range(NCORES):
        sl = slice(c * SH, (c + 1) * SH)
        H_c = np.zeros((SHP, NV), fdt)
        H_c[:SH] = H[sl]
        X_c = np.zeros((SHP, EMB), fdt)
        X_c[:SH] = X[sl]
        WoT_c = np.zeros((EMB, SHP), odt)
        WoT_c[:, :SH] = W_out_w[sl].T
        Wob_c = np.zeros((SHP,), np.float32)
        Wob_c[:SH] = W_out_b[sl]
        Wob_c = np.ascontiguousarray(Wob_c.reshape(KT, 128).T)
        in_maps.append(
            {
                "h_mat": H_c,
                "x_mat": X_c,
                "wout_t": WoT_c,
                "wout_b": Wob_c,
                "phe": phe,
                "wcw_t": WcwT,
                "wcw_b": bcw,
                "wih_t": WihT,
                "gates_bias": gates_bias,
                "whh_t": WhhT,
                "bhn": bhn,
            }
        )

    nc = _get_program()
    res = bass_utils.run_bass_kernel_spmd(
        nc,
        in_maps,
        core_ids=list(range(NCORES)),
        trace=bool(int(os.environ.get("KERNEL_TRACE", "0"))),
    )
    kernel.last_results = res
    out = np.concatenate(
        [np.asarray(res.results[c]["preds"], np.float32)[:SH] for c in range(NCORES)],
        axis=0,
    )
    return out


if __name__ == "__main__":
    np.random.seed(0)
    ins = {
        "phenotype_embs": np.random.randn(NTP, HID).astype(np.float32) * 0.02,
        "H": (np.random.rand(NC, NV) < 0.05).astype(np.float32),
        "X": np.random.randn(NC, EMB).astype(np.float32) * 0.02,
        "W_context_w": np.random.randn(HID, HID * NTP).astype(np.float32) * 0.02,
        "W_context_b": np.random.randn(HID).astype(np.float32) * 0.02,
        "gru_W_ih": np.random.randn(3 * HID, EMB).astype(np.float32) * 0.02,
        "gru_W_hh": np.random.randn(3 * HID, HID).astype(np.float32) * 0.02,
        "gru_b_ih": np.random.randn(3 * HID).astype(np.float32) * 0.02,
        "gru_b_hh": np.random.randn(3 * HID).astype(np.float32) * 0.02,
        "W_out_w": np.random.randn(NC, HID).astype(np.float32) * 0.02,
        "W_out_b": np.random.randn(NC).astype(np.float32) * 0.02,
        "num_visits": NV,
    }
    out = kernel(**ins)
    print("out", out.shape, out.dtype, out[:2, :4])


# revision 4
# speedup vs baseline: 2.2665x; 2.2665x over previous
"""Trainium2 Bass kernel for nn_Decoder (GRU decoder over phenotype codes).

Computation (see module docstring history / reference.py):
  h0 = W_cw @ phe_flat + b_cw                       (256,)
  G  = H.T @ X                                      (100, 256)   big GEMM, sharded over codes
  emb[t] = relu(G[t-1]), emb[0] = 0
  gates_x = emb @ W_ih.T + b_ih                     (100, 768)
  GRU scan over 100 steps (hidden 256)
  preds = sigmoid(hs @ W_out.T + W_out_b).T         (50000, 100)

Sharding: codes (50000) split over 8 cores (6250 -> padded 6272 each).
Front GEMM partial per core -> AllReduce(100x256) -> replicated GRU scan ->
sharded output GEMM + sigmoid.

All large tensors are pre-laid-out on the host into [128, ...] partition-major
form so each DMA is one contiguous descriptor per partition.
"""

import os
import sys

import numpy as np

sys.path.insert(0, "/opt/trn_rl_repo")

import ml_dtypes  # noqa: E402

import concourse.bass as bass  # noqa: E402
import concourse.mybir as mybir  # noqa: E402
import concourse.tile as tile  # noqa: E402
from concourse import bacc, bass_utils  # noqa: E402
from concourse.masks import make_identity  # noqa: E402

HID = 256
NTP = 8
EMB = 256
NC = 50000
NV = 100
NCORES = 8
SH = NC // NCORES  # 6250
KT = (SH + 127) // 128  # 49 k-tiles per shard
SHP = KT * 128  # 6272 padded codes per core

F32 = mybir.dt.float32
BF16 = mybir.dt.bfloat16

# ---- dtype knobs ----
DT_FRONT = BF16  # H, X, front GEMM
DT_SCAN = BF16  # gates GEMM, W_hh, h state, GX, identity
DT_OUT = BF16  # W_out, output GEMM
SPLIT_HH = True  # W_hh as bf16 hi+lo pair (fp32r-like accuracy, 2 passes)
SPLIT_CW = False  # same for the h0 GEMM weights
NPASS_HH = 2 if SPLIT_HH else 1
NPASS_CW = 2 if SPLIT_CW else 1


def _npdt(dt):
    return np.float32 if dt == F32 else ml_dtypes.bfloat16


def build_program():
    nc = bacc.Bacc(
        "TRN2",
        target_bir_lowering=False,
        debug=False,
        enable_asserts=False,
        num_devices=NCORES,
    )

    # ---- DRAM I/O (per core); all pre-laid-out [128, ...] on host ----
    H_d = nc.dram_tensor("h_mat", [128, KT, NV], DT_FRONT, kind="ExternalInput")
    X_d = nc.dram_tensor("x_mat", [128, KT, EMB], DT_FRONT, kind="ExternalInput")
    WoT_d = nc.dram_tensor("wout_t", [128, 2, SHP], DT_OUT, kind="ExternalInput")
    Wob_d = nc.dram_tensor("wout_b", [128, KT], F32, kind="ExternalInput")
    phe_d = nc.dram_tensor("phe", [128, 16], F32, kind="ExternalInput")
    WcwT_d = nc.dram_tensor(
        "wcw_t", [128, NPASS_CW, 16, HID], DT_SCAN, kind="ExternalInput"
    )
    bcw_d = nc.dram_tensor("wcw_b", [128, 2], F32, kind="ExternalInput")
    WihT_d = nc.dram_tensor("wih_t", [128, 2, 3 * HID], DT_SCAN, kind="ExternalInput")
    bih_d = nc.dram_tensor("gates_bias", [1, 3 * HID], DT_SCAN, kind="ExternalInput")
    WhhT_d = nc.dram_tensor(
        "whh_t", [128, NPASS_HH, 2, 3 * HID], DT_SCAN, kind="ExternalInput"
    )
    bhn_d = nc.dram_tensor("bhn", [128, 2], DT_SCAN, kind="ExternalInput")
    out_d = nc.dram_tensor("preds", [SHP, NV], F32, kind="ExternalOutput")

    with tile.TileContext(nc) as tc:
        with (
            tc.tile_pool(name="const", bufs=1) as cpool,
            tc.tile_pool(name="big", bufs=1) as bigpool,
            tc.tile_pool(name="work", bufs=2) as wpool,
            tc.tile_pool(name="dram", bufs=1, space="DRAM") as dpool,
        ):
            # ---- persistent SBUF tiles ----
            phe_sb = cpool.tile([128, 16], F32)
            WcwT_sb = cpool.tile([128, NPASS_CW, 16, HID], DT_SCAN)
            bcw_sb = cpool.tile([128, 2], F32)
            WihT_sb = cpool.tile([128, 2, 3 * HID], DT_SCAN)
            bih_sb = cpool.tile([1, 3 * HID], DT_SCAN)
            WhhT_sb = cpool.tile([128, NPASS_HH, 2, 3 * HID], DT_SCAN)
            bhn_sb = cpool.tile([128, 2], DT_SCAN)
            Wob_sb = cpool.tile([128, KT], F32)
            ID_sb = cpool.tile([128, 128], DT_SCAN)
            ones_sb = cpool.tile([1, NV], DT_SCAN)
            H_sb = bigpool.tile([128, KT, NV], DT_FRONT)
            X_sb = bigpool.tile([128, KT, EMB], DT_FRONT)
            WoT_sb = bigpool.tile([128, 2, SHP], DT_OUT)
            G_sb = cpool.tile([NV, EMB], F32)
            Gr_sb = cpool.tile([NV, EMB], DT_SCAN)
            embT_sb = cpool.tile([128, 2, NV], DT_SCAN)
            GX_sb = cpool.tile([128, NV, 4], DT_SCAN)
            GXN_sb = cpool.tile([128, NV, 2], DT_SCAN)
            hs_sb = cpool.tile([128, 2, NV], DT_SCAN)
            h0_sb = cpool.tile([128, 2], DT_SCAN)

            ar_in = dpool.tile([NV, EMB], F32)
            ar_out = dpool.tile([NV, EMB], F32)

            # ---- small DMAs + constants ----
            nc.sync.dma_start(phe_sb[:], phe_d[:])
            nc.sync.dma_start(WcwT_sb[:], WcwT_d[:])
            nc.sync.dma_start(bcw_sb[:], bcw_d[:])
            nc.sync.dma_start(WihT_sb[:], WihT_d[:])
            nc.sync.dma_start(bih_sb[:], bih_d[:])
            nc.sync.dma_start(WhhT_sb[:], WhhT_d[:])
            nc.sync.dma_start(bhn_sb[:], bhn_d[:])
            nc.sync.dma_start(Wob_sb[:], Wob_d[:])
            make_identity(nc, ID_sb[:])
            nc.gpsimd.memset(ones_sb[:], 1.0)
            nc.gpsimd.memset(embT_sb[:], 0.0)

            # ---- big DMAs ----
            nc.sync.dma_start(H_sb[:], H_d[:])
            nc.sync.dma_start(X_sb[:], X_d[:])
            nc.sync.dma_start(WoT_sb[:], WoT_d[:])

            # ---- h0 = W_cw @ phe + b_cw ----
            with tc.tile_pool(name="ps_h0", bufs=1, space="PSUM") as pp:
                ps_h0 = pp.tile([128, 2], F32)
                phe_cast = wpool.tile([128, 16], DT_SCAN, tag="phec")
                if DT_SCAN != F32:
                    nc.vector.tensor_copy(phe_cast[:], phe_sb[:])
                    phe_use = phe_cast
                else:
                    phe_use = phe_sb
                for m in range(2):
                    first = True
                    for p in range(NPASS_CW):
                        for k in range(16):
                            nc.tensor.matmul(
                                ps_h0[:, m : m + 1],
                                WcwT_sb[:, p, k, m * 128 : (m + 1) * 128],
                                phe_use[:, k : k + 1],
                                start=first,
                                stop=(p == NPASS_CW - 1 and k == 15),
                            )
                            first = False
                nc.vector.tensor_add(h0_sb[:], ps_h0[:], bcw_sb[:])

            # ---- front GEMM: G_partial = H_c.T @ X_c  (100 x 256) ----
            with tc.tile_pool(name="ps_g", bufs=1, space="PSUM") as pp:
                ps_g = pp.tile([NV, EMB], F32)
                for k in range(KT):
                    nc.tensor.matmul(
                        ps_g[:],
                        H_sb[:, k, :],
                        X_sb[:, k, :],
                        start=(k == 0),
                        stop=(k == KT - 1),
                    )
                Gp_sb = wpool.tile([NV, EMB], F32)
                nc.vector.tensor_copy(Gp_sb[:], ps_g[:])

            # ---- AllReduce the partial G ----
            nc.sync.dma_start(ar_in[:], Gp_sb[:])
            nc.gpsimd.collective_compute(
                "AllReduce",
                mybir.AluOpType.add,
                replica_groups=[list(range(NCORES))],
                ins=[ar_in.opt()],
                outs=[ar_out.opt()],
            )
            nc.sync.dma_start(G_sb[:], ar_out[:])

            # ---- emb.T = shifted relu(G).T  (256 x 100, col 0 = zeros) ----
            nc.vector.tensor_scalar(Gr_sb[:], G_sb[:], 0.0, None, mybir.AluOpType.max)
            with tc.tile_pool(name="ps_tr", bufs=2, space="PSUM") as pp:
                for c in range(2):
                    ps_tr = pp.tile([128, NV - 1], DT_SCAN, tag="tr")
                    nc.tensor.transpose(
                        ps_tr[:],
                        Gr_sb[0 : NV - 1, c * 128 : (c + 1) * 128],
                        ID_sb[0 : NV - 1, 0 : NV - 1],
                    )
                    nc.vector.tensor_copy(embT_sb[:, c, 1:NV], ps_tr[:])

            # ---- gates_x GEMM -> GX (rz, interleaved) and GXN (n) ----
            with tc.tile_pool(name="ps_gx", bufs=3, space="PSUM") as pp:
                for m in range(6):
                    ps_gx = pp.tile([128, NV], F32, tag="gx")
                    for kc in range(2):
                        nc.tensor.matmul(
                            ps_gx[:],
                            WihT_sb[:, kc, m * 128 : (m + 1) * 128],
                            embT_sb[:, kc, :],
                            start=(kc == 0),
                            stop=False,
                        )
                    nc.tensor.matmul(
                        ps_gx[:],
                        bih_sb[0:1, m * 128 : (m + 1) * 128],
                        ones_sb[0:1, :],
                        start=False,
                        stop=True,
                    )
                    if m < 4:
                        nc.vector.tensor_copy(GX_sb[:, :, m], ps_gx[:])
                    else:
                        nc.vector.tensor_copy(GXN_sb[:, :, m - 4], ps_gx[:])

            # ---- GRU scan ----
            with (
                tc.tile_pool(name="ps_scan", bufs=2, space="PSUM") as pp,
                tc.tile_pool(name="scan_t", bufs=3) as sp,
            ):
                for t in range(NV):
                    hcur = h0_sb if t == 0 else hs_sb[:, :, t - 1]
                    ps = pp.tile([128, 6], F32, tag="scan")
                    # seed psum with gate biases / rz inputs (sets has_written)
                    nc.tensor.matmul(
                        ps[:, 0:4], ID_sb[:], GX_sb[:, t, :], start=True, stop=False
                    )
                    nc.tensor.matmul(
                        ps[:, 4:6], ID_sb[:], bhn_sb[:], start=True, stop=False
                    )
                    for m in range(6):
                        for p in range(NPASS_HH):
                            for kc in range(2):
                                nc.tensor.matmul(
                                    ps[:, m : m + 1],
                                    WhhT_sb[:, p, kc, m * 128 : (m + 1) * 128],
                                    hcur[:, kc : kc + 1],
                                    start=False,
                                    stop=(m == 5 and p == NPASS_HH - 1 and kc == 1),
                                )
                    rz = sp.tile([128, 4], DT_SCAN, tag="rz")
                    nc.scalar.activation(
                        rz[:], ps[:, 0:4], mybir.ActivationFunctionType.Sigmoid
                    )
                    t1 = sp.tile([128, 2], DT_SCAN, tag="t1")
                    nc.vector.tensor_mul(t1[:], rz[:, 0:2], ps[:, 4:6])
                    t2 = sp.tile([128, 2], DT_SCAN, tag="t2")
                    nc.vector.tensor_add(t2[:], t1[:], GXN_sb[:, t, :])
                    n_t = sp.tile([128, 2], DT_SCAN, tag="n")
                    nc.scalar.activation(
                        n_t[:], t2[:], mybir.ActivationFunctionType.Tanh
                    )
                    w_t = sp.tile([128, 2], DT_SCAN, tag="w")
                    nc.vector.tensor_scalar(
                        w_t[:],
                        rz[:, 2:4],
                        -1.0,
                        1.0,
                        mybir.AluOpType.mult,
                        mybir.AluOpType.add,
                    )
                    u_t = sp.tile([128, 2], DT_SCAN, tag="u")
                    nc.vector.tensor_mul(u_t[:], rz[:, 2:4], hcur)
                    # h = (n * w) + u, fused per column
                    for c in range(2):
                        nc.vector.scalar_tensor_tensor(
                            hs_sb[:, c : c + 1, t],
                            n_t[:, c : c + 1],
                            w_t[:, c : c + 1],
                            u_t[:, c : c + 1],
                            mybir.AluOpType.mult,
                            mybir.AluOpType.add,
                        )

            # ---- output GEMM + sigmoid + store ----
            with (
                tc.tile_pool(name="ps_out", bufs=4, space="PSUM") as pp,
                tc.tile_pool(name="pred", bufs=4) as op,
            ):
                for m in range(KT):
                    ps_o = pp.tile([128, NV], F32, tag="o")
                    for kc in range(2):
                        nc.tensor.matmul(
                            ps_o[:],
                            WoT_sb[:, kc, m * 128 : (m + 1) * 128],
                            hs_sb[:, kc, :],
                            start=(kc == 0),
                            stop=(kc == 1),
                        )
                    pred = op.tile([128, NV], F32, tag="p")
                    nc.scalar.activation(
                        pred[:],
                        ps_o[:],
                        mybir.ActivationFunctionType.Sigmoid,
                        bias=Wob_sb[:, m : m + 1],
                    )
                    nc.sync.dma_start(out_d[m * 128 : (m + 1) * 128, :], pred[:])

    nc.compile()
    return nc


_PROG = None


def _get_program():
    global _PROG
    if _PROG is None:
        _PROG = build_program()
    return _PROG


def _pack2(v):
    # (256,) -> (128, 2) chunk-major: col c = v[128c:128c+128]
    return np.ascontiguousarray(v.reshape(2, 128).T)


def _chunked(a, dt):
    """(C*128, F...) row-major -> [128, C, F...] partition-major, contiguous."""
    c = a.shape[0] // 128
    out = np.ascontiguousarray(
        a.reshape((c, 128) + a.shape[1:]).swapaxes(0, 1).astype(dt)
    )
    return out


def _split_hi_lo(a):
    hi = a.astype(ml_dtypes.bfloat16)
    lo = (a - hi.astype(np.float32)).astype(ml_dtypes.bfloat16)
    return np.stack([hi, lo], axis=1)  # [128, 2, ...]


def kernel(
    phenotype_embs,
    H,
    X,
    W_context_w,
    W_context_b,
    gru_W_ih,
    gru_W_hh,
    gru_b_ih,
    gru_b_hh,
    W_out_w,
    W_out_b,
    num_visits,
):
    assert int(num_visits) == NV
    fdt = _npdt(DT_FRONT)
    sdt = _npdt(DT_SCAN)
    odt = _npdt(DT_OUT)

    H = np.asarray(H, np.float32)
    X = np.asarray(X, np.float32)
    W_out_w = np.asarray(W_out_w, np.float32)
    W_out_b = np.asarray(W_out_b, np.float32)

    # replicated small tensors
    phe = np.ascontiguousarray(
        np.asarray(phenotype_embs, np.float32).reshape(-1).reshape(16, 128).T
    )
    WcwT = _chunked(np.ascontiguousarray(np.asarray(W_context_w, np.float32).T), np.float32)
    # [128, 16, 256]
    if SPLIT_CW:
        WcwT = np.ascontiguousarray(
            _split_hi_lo(WcwT).reshape(128, 2, 16, HID)
        )
    else:
        WcwT = WcwT.astype(sdt).reshape(128, 1, 16, HID)
    bcw = _pack2(np.asarray(W_context_b, np.float32))
    WihT = _chunked(np.ascontiguousarray(np.asarray(gru_W_ih, np.float32).T), sdt)
    b_ih = np.asarray(gru_b_ih, np.float32)
    b_hh = np.asarray(gru_b_hh, np.float32)
    gates_bias = b_ih.copy()
    gates_bias[: 2 * HID] += b_hh[: 2 * HID]
    gates_bias = gates_bias.reshape(1, -1).astype(sdt)
    WhhT_f = _chunked(
        np.ascontiguousarray(np.asarray(gru_W_hh, np.float32).T), np.float32
    )  # [128, 2, 768]
    if SPLIT_HH:
        WhhT = np.ascontiguousarray(_split_hi_lo(WhhT_f))  # [128, 2(hi/lo), 2, 768]
    else:
        WhhT = np.ascontiguousarray(WhhT_f.astype(sdt).reshape(128, 1, 2, 3 * HID))
    bhn = _pack2(b_hh[2 * HID :]).astype(sdt)

    in_maps = []
    for c in range(NCORES):
        sl = slice(c * SH, (c + 1) * SH)
        H_c = np.zeros((SHP, NV), np.float32)
        H_c[:SH] = H[sl]
        X_c = np.zeros((SHP, EMB), np.float32)
        X_c[:SH] = X[sl]
        WoT_c = np.zeros((EMB, SHP), np.float32)
        WoT_c[:, :SH] = W_out_w[sl].T
        Wob_c = np.zeros((SHP,), np.float32)
        Wob_c[:SH] = W_out_b[sl]
        in_maps.append(
            {
                "h_mat": _chunked(H_c, fdt),
                "x_mat": _chunked(X_c, fdt),
                "wout_t": _chunked(WoT_c, odt),
                "wout_b": np.ascontiguousarray(Wob_c.reshape(KT, 128).T),
                "phe": phe,
                "wcw_t": WcwT,
                "wcw_b": bcw,
                "wih_t": WihT,
                "gates_bias": gates_bias,
                "whh_t": WhhT,
                "bhn": bhn,
            }
        )

    nc = _get_program()
    res = bass_utils.run_bass_kernel_spmd(
        nc,
        in_maps,
        core_ids=list(range(NCORES)),
        trace=bool(int(os.environ.get("KERNEL_TRACE", "0"))),
    )
    kernel.last_results = res
    out = np.concatenate(
        [np.asarray(res.results[c]["preds"], np.float32)[:SH] for c in range(NCORES)],
        axis=0,
    )
    return out


if __name__ == "__main__":
    np.random.seed(0)
    ins = {
        "phenotype_embs": np.random.randn(NTP, HID).astype(np.float32) * 0.02,
        "H": (np.random.rand(NC, NV) < 0.05).astype(np.float32),
        "X": np.random.randn(NC, EMB).astype(np.float32) * 0.02,
        "W_context_w": np.random.randn(HID, HID * NTP).astype(np.float32) * 0.02,
        "W_context_b": np.random.randn(HID).astype(np.float32) * 0.02,
        "gru_W_ih": np.random.randn(3 * HID, EMB).astype(np.float32) * 0.02,
        "gru_W_hh": np.random.randn(3 * HID, HID).astype(np.float32) * 0.02,
        "gru_b_ih": np.random.randn(3 * HID).astype(np.float32) * 0.02,
        "gru_b_hh": np.random.randn(3 * HID).astype(np.float32) * 0.02,
        "W_out_w": np.random.randn(NC, HID).astype(np.float32) * 0.02,
        "W_out_b": np.random.randn(NC).astype(np.float32) * 0.02,
        "num_visits": NV,
    }
    out = kernel(**ins)
    print("out", out.shape, out.dtype, out[:2, :4])


# revision 7
# speedup vs baseline: 2.3424x; 1.0335x over previous
"""Trainium2 Bass kernel for nn_Decoder (GRU decoder over phenotype codes).

Computation (see module docstring history / reference.py):
  h0 = W_cw @ phe_flat + b_cw                       (256,)
  G  = H.T @ X                                      (100, 256)   big GEMM, sharded over codes
  emb[t] = relu(G[t-1]), emb[0] = 0
  gates_x = emb @ W_ih.T + b_ih                     (100, 768)
  GRU scan over 100 steps (hidden 256)
  preds = sigmoid(hs @ W_out.T + W_out_b).T         (50000, 100)

Sharding: codes (50000) split over 8 cores (6250 -> padded 6272 each).
Front GEMM partial per core -> AllReduce(100x256) -> replicated GRU scan ->
sharded output GEMM + sigmoid.

All large tensors are pre-laid-out on the host into [128, ...] partition-major
form so each DMA is one contiguous descriptor per partition.
"""

import os
import sys

import numpy as np

sys.path.insert(0, "/opt/trn_rl_repo")

import ml_dtypes  # noqa: E402

import concourse.bass as bass  # noqa: E402
import concourse.mybir as mybir  # noqa: E402
import concourse.tile as tile  # noqa: E402
from concourse import bacc, bass_utils  # noqa: E402
from concourse.masks import make_identity  # noqa: E402

HID = 256
NTP = 8
EMB = 256
NC = 50000
NV = 100
NCORES = 8
SH = NC // NCORES  # 6250
KT = (SH + 127) // 128  # 49 k-tiles per shard
SHP = KT * 128  # 6272 padded codes per core

F32 = mybir.dt.float32
BF16 = mybir.dt.bfloat16

# ---- dtype knobs ----
DT_FRONT = BF16  # H, X, front GEMM
DT_SCAN = BF16  # gates GEMM, W_hh, h state, GX, identity
DT_OUT = BF16  # W_out, output GEMM
SPLIT_HH = True  # W_hh as bf16 hi+lo pair (fp32r-like accuracy, 2 passes)
SPLIT_CW = False  # same for the h0 GEMM weights
NPASS_HH = 2 if SPLIT_HH else 1
NPASS_CW = 2 if SPLIT_CW else 1


def _npdt(dt):
    return np.float32 if dt == F32 else ml_dtypes.bfloat16


def build_program():
    nc = bacc.Bacc(
        "TRN2",
        target_bir_lowering=False,
        debug=False,
        enable_asserts=False,
        num_devices=NCORES,
    )

    # ---- DRAM I/O (per core); all pre-laid-out [128, ...] on host ----
    H_d = nc.dram_tensor("h_mat", [128, KT, NV], DT_FRONT, kind="ExternalInput")
    X_d = nc.dram_tensor("x_mat", [128, KT, EMB], DT_FRONT, kind="ExternalInput")
    WoT_d = nc.dram_tensor("wout_t", [128, 2, SHP], DT_OUT, kind="ExternalInput")
    Wob_d = nc.dram_tensor("wout_b", [128, KT], F32, kind="ExternalInput")
    phe_d = nc.dram_tensor("phe", [128, 16], F32, kind="ExternalInput")
    WcwT_d = nc.dram_tensor(
        "wcw_t", [128, NPASS_CW, 16, HID], DT_SCAN, kind="ExternalInput"
    )
    bcw_d = nc.dram_tensor("wcw_b", [128, 2], F32, kind="ExternalInput")
    WihT_d = nc.dram_tensor("wih_t", [128, 2, 3 * HID], DT_SCAN, kind="ExternalInput")
    bih_d = nc.dram_tensor("gates_bias", [1, 3 * HID], DT_SCAN, kind="ExternalInput")
    WhhT_d = nc.dram_tensor(
        "whh_t", [128, NPASS_HH, 2, 3 * HID], DT_SCAN, kind="ExternalInput"
    )
    bhn_d = nc.dram_tensor("bhn", [128, 2], DT_SCAN, kind="ExternalInput")
    out_d = nc.dram_tensor("preds", [SHP, NV], F32, kind="ExternalOutput")

    with tile.TileContext(nc) as tc:
        with (
            tc.tile_pool(name="const", bufs=1) as cpool,
            tc.tile_pool(name="big", bufs=1) as bigpool,
            tc.tile_pool(name="work", bufs=2) as wpool,
            tc.tile_pool(name="dram", bufs=1, space="DRAM") as dpool,
        ):
            # ---- persistent SBUF tiles ----
            phe_sb = cpool.tile([128, 16], F32)
            WcwT_sb = cpool.tile([128, NPASS_CW, 16, HID], DT_SCAN)
            bcw_sb = cpool.tile([128, 2], F32)
            WihT_sb = cpool.tile([128, 2, 3 * HID], DT_SCAN)
            bih_sb = cpool.tile([1, 3 * HID], DT_SCAN)
            WhhT_sb = cpool.tile([128, NPASS_HH, 2, 3 * HID], DT_SCAN)
            bhn_sb = cpool.tile([128, 2], DT_SCAN)
            Wob_sb = cpool.tile([128, KT], F32)
            ID_sb = cpool.tile([128, 128], DT_SCAN)
            ones_sb = cpool.tile([1, NV], DT_SCAN)
            H_sb = bigpool.tile([128, KT, NV], DT_FRONT)
            X_sb = bigpool.tile([128, KT, EMB], DT_FRONT)
            WoT_sb = bigpool.tile([128, 2, SHP], DT_OUT)
            G_sb = cpool.tile([NV, EMB], F32)
            Gr_sb = cpool.tile([NV, EMB], DT_SCAN)
            embT_sb = cpool.tile([128, 2, NV], DT_SCAN)
            GX_sb = cpool.tile([128, NV, 4], DT_SCAN)
            GXN_sb = cpool.tile([128, NV, 2], DT_SCAN)
            hs_sb = cpool.tile([128, 2, NV], DT_SCAN)
            h0_sb = cpool.tile([128, 2], DT_SCAN)

            ar_in = dpool.tile([NV, EMB], F32)
            ar_out = dpool.tile([NV, EMB], F32)

            # ---- small DMAs + constants ----
            nc.sync.dma_start(phe_sb[:], phe_d[:])
            nc.sync.dma_start(WcwT_sb[:], WcwT_d[:])
            nc.sync.dma_start(bcw_sb[:], bcw_d[:])
            nc.sync.dma_start(WihT_sb[:], WihT_d[:])
            nc.sync.dma_start(bih_sb[:], bih_d[:])
            nc.sync.dma_start(WhhT_sb[:], WhhT_d[:])
            nc.sync.dma_start(bhn_sb[:], bhn_d[:])
            nc.sync.dma_start(Wob_sb[:], Wob_d[:])
            make_identity(nc, ID_sb[:])
            nc.gpsimd.memset(ones_sb[:], 1.0)
            nc.gpsimd.memset(embT_sb[:], 0.0)

            # ---- big DMAs (chunked so they spread across DMA engines) ----
            bounds = [0, 7, 14, 21, 28, 35, 42, KT]
            for i in range(len(bounds) - 1):
                ksl = slice(bounds[i], bounds[i + 1])
                nc.sync.dma_start(H_sb[:, ksl, :], H_d[:, ksl, :])
                nc.sync.dma_start(X_sb[:, ksl, :], X_d[:, ksl, :])
            for kc in range(2):
                for i in range(8):
                    nsl = slice(i * (SHP // 8), (i + 1) * (SHP // 8))
                    nc.sync.dma_start(WoT_sb[:, kc, nsl], WoT_d[:, kc, nsl])

            # ---- h0 = W_cw @ phe + b_cw ----
            with tc.tile_pool(name="ps_h0", bufs=1, space="PSUM") as pp:
                ps_h0 = pp.tile([128, 2], F32)
                phe_cast = wpool.tile([128, 16], DT_SCAN, tag="phec")
                if DT_SCAN != F32:
                    nc.vector.tensor_copy(phe_cast[:], phe_sb[:])
                    phe_use = phe_cast
                else:
                    phe_use = phe_sb
                for m in range(2):
                    first = True
                    for p in range(NPASS_CW):
                        for k in range(16):
                            nc.tensor.matmul(
                                ps_h0[:, m : m + 1],
                                WcwT_sb[:, p, k, m * 128 : (m + 1) * 128],
                                phe_use[:, k : k + 1],
                                start=first,
                                stop=(p == NPASS_CW - 1 and k == 15),
                            )
                            first = False
                nc.vector.tensor_add(h0_sb[:], ps_h0[:], bcw_sb[:])

            # ---- front GEMM: G_partial = H_c.T @ X_c  (100 x 256) ----
            with tc.tile_pool(name="ps_g", bufs=1, space="PSUM") as pp:
                ps_g = pp.tile([NV, EMB], F32)
                for k in range(KT):
                    nc.tensor.matmul(
                        ps_g[:],
                        H_sb[:, k, :],
                        X_sb[:, k, :],
                        start=(k == 0),
                        stop=(k == KT - 1),
                    )
                Gp_sb = wpool.tile([NV, EMB], F32)
                nc.vector.tensor_copy(Gp_sb[:], ps_g[:])

            # ---- AllReduce the partial G ----
            nc.sync.dma_start(ar_in[:], Gp_sb[:])
            nc.gpsimd.collective_compute(
                "AllReduce",
                mybir.AluOpType.add,
                replica_groups=[list(range(NCORES))],
                ins=[ar_in.opt()],
                outs=[ar_out.opt()],
            )
            nc.sync.dma_start(G_sb[:], ar_out[:])

            # ---- emb.T = shifted relu(G).T  (256 x 100, col 0 = zeros) ----
            nc.vector.tensor_scalar(Gr_sb[:], G_sb[:], 0.0, None, mybir.AluOpType.max)
            with tc.tile_pool(name="ps_tr", bufs=2, space="PSUM") as pp:
                for c in range(2):
                    ps_tr = pp.tile([128, NV - 1], DT_SCAN, tag="tr")
                    nc.tensor.transpose(
                        ps_tr[:],
                        Gr_sb[0 : NV - 1, c * 128 : (c + 1) * 128],
                        ID_sb[0 : NV - 1, 0 : NV - 1],
                    )
                    nc.vector.tensor_copy(embT_sb[:, c, 1:NV], ps_tr[:])

            # ---- gates_x GEMM -> GX (rz, interleaved) and GXN (n) ----
            with tc.tile_pool(name="ps_gx", bufs=3, space="PSUM") as pp:
                for m in range(6):
                    ps_gx = pp.tile([128, NV], F32, tag="gx")
                    for kc in range(2):
                        nc.tensor.matmul(
                            ps_gx[:],
                            WihT_sb[:, kc, m * 128 : (m + 1) * 128],
                            embT_sb[:, kc, :],
                            start=(kc == 0),
                            stop=False,
                        )
                    nc.tensor.matmul(
                        ps_gx[:],
                        bih_sb[0:1, m * 128 : (m + 1) * 128],
                        ones_sb[0:1, :],
                        start=False,
                        stop=True,
                    )
                    if m < 4:
                        nc.vector.tensor_copy(GX_sb[:, :, m], ps_gx[:])
                    else:
                        nc.vector.tensor_copy(GXN_sb[:, :, m - 4], ps_gx[:])

            # ---- GRU scan ----
            with (
                tc.tile_pool(name="ps_scan", bufs=2, space="PSUM") as pp,
                tc.tile_pool(name="scan_t", bufs=3) as sp,
            ):
                for t in range(NV):
                    hcur = h0_sb if t == 0 else hs_sb[:, :, t - 1]
                    ps_rz = pp.tile([128, 4], F32, tag="scan_rz")
                    ps_n = pp.tile([128, 2], F32, tag="scan_n")
                    # seed psum with gate biases / rz inputs (sets has_written)
                    nc.tensor.matmul(
                        ps_rz[:], ID_sb[:], GX_sb[:, t, :], start=True, stop=False
                    )
                    nc.tensor.matmul(
                        ps_n[:], ID_sb[:], bhn_sb[:], start=True, stop=False
                    )
                    # rz passes first so sigmoid overlaps the n-gate passes
                    for m in range(4):
                        for p in range(NPASS_HH):
                            for kc in range(2):
                                nc.tensor.matmul(
                                    ps_rz[:, m : m + 1],
                                    WhhT_sb[:, p, kc, m * 128 : (m + 1) * 128],
                                    hcur[:, kc : kc + 1],
                                    start=False,
                                    stop=(m == 3 and p == NPASS_HH - 1 and kc == 1),
                                )
                    for m in range(4, 6):
                        for p in range(NPASS_HH):
                            for kc in range(2):
                                nc.tensor.matmul(
                                    ps_n[:, m - 4 : m - 3],
                                    WhhT_sb[:, p, kc, m * 128 : (m + 1) * 128],
                                    hcur[:, kc : kc + 1],
                                    start=False,
                                    stop=(m == 5 and p == NPASS_HH - 1 and kc == 1),
                                )
                    rz = sp.tile([128, 4], DT_SCAN, tag="rz")
                    nc.scalar.activation(
                        rz[:], ps_rz[:], mybir.ActivationFunctionType.Sigmoid
                    )
                    t1 = sp.tile([128, 2], DT_SCAN, tag="t1")
                    nc.vector.tensor_mul(t1[:], rz[:, 0:2], ps_n[:])
                    t2 = sp.tile([128, 2], DT_SCAN, tag="t2")
                    nc.vector.tensor_add(t2[:], t1[:], GXN_sb[:, t, :])
                    n_t = sp.tile([128, 2], DT_SCAN, tag="n")
                    nc.scalar.activation(
                        n_t[:], t2[:], mybir.ActivationFunctionType.Tanh
                    )
                    w_t = sp.tile([128, 2], DT_SCAN, tag="w")
                    nc.vector.tensor_scalar(
                        w_t[:],
                        rz[:, 2:4],
                        -1.0,
                        1.0,
                        mybir.AluOpType.mult,
                        mybir.AluOpType.add,
                    )
                    u_t = sp.tile([128, 2], DT_SCAN, tag="u")
                    nc.vector.tensor_mul(u_t[:], rz[:, 2:4], hcur)
                    v_t = sp.tile([128, 2], DT_SCAN, tag="v")
                    nc.vector.tensor_mul(v_t[:], n_t[:], w_t[:])
                    nc.vector.tensor_add(hs_sb[:, :, t], v_t[:], u_t[:])

            # ---- output GEMM + sigmoid + store ----
            with (
                tc.tile_pool(name="ps_out", bufs=6, space="PSUM") as pp,
                tc.tile_pool(name="pred", bufs=6) as op,
            ):
                for m in range(KT):
                    ps_o = pp.tile([128, NV], F32, tag="o")
                    for kc in range(2):
                        nc.tensor.matmul(
                            ps_o[:],
                            WoT_sb[:, kc, m * 128 : (m + 1) * 128],
                            hs_sb[:, kc, :],
                            start=(kc == 0),
                            stop=(kc == 1),
                        )
                    pred = op.tile([128, NV], F32, tag="p")
                    nc.scalar.activation(
                        pred[:],
                        ps_o[:],
                        mybir.ActivationFunctionType.Sigmoid,
                        bias=Wob_sb[:, m : m + 1],
                    )
                    nc.sync.dma_start(out_d[m * 128 : (m + 1) * 128, :], pred[:])

    nc.compile()
    return nc


_PROG = None


def _get_program():
    global _PROG
    if _PROG is None:
        _PROG = build_program()
    return _PROG


def _pack2(v):
    # (256,) -> (128, 2) chunk-major: col c = v[128c:128c+128]
    return np.ascontiguousarray(v.reshape(2, 128).T)


def _chunked(a, dt):
    """(C*128, F...) row-major -> [128, C, F...] partition-major, contiguous."""
    c = a.shape[0] // 128
    out = np.ascontiguousarray(
        a.reshape((c, 128) + a.shape[1:]).swapaxes(0, 1).astype(dt)
    )
    return out


def _split_hi_lo(a):
    hi = a.astype(ml_dtypes.bfloat16)
    lo = (a - hi.astype(np.float32)).astype(ml_dtypes.bfloat16)
    return np.stack([hi, lo], axis=1)  # [128, 2, ...]


def kernel(
    phenotype_embs,
    H,
    X,
    W_context_w,
    W_context_b,
    gru_W_ih,
    gru_W_hh,
    gru_b_ih,
    gru_b_hh,
    W_out_w,
    W_out_b,
    num_visits,
):
    assert int(num_visits) == NV
    fdt = _npdt(DT_FRONT)
    sdt = _npdt(DT_SCAN)
    odt = _npdt(DT_OUT)

    H = np.asarray(H, np.float32)
    X = np.asarray(X, np.float32)
    W_out_w = np.asarray(W_out_w, np.float32)
    W_out_b = np.asarray(W_out_b, np.float32)

    # replicated small tensors
    phe = np.ascontiguousarray(
        np.asarray(phenotype_embs, np.float32).reshape(-1).reshape(16, 128).T
    )
    WcwT = _chunked(np.ascontiguousarray(np.asarray(W_context_w, np.float32).T), np.float32)
    # [128, 16, 256]
    if SPLIT_CW:
        WcwT = np.ascontiguousarray(
            _split_hi_lo(WcwT).reshape(128, 2, 16, HID)
        )
    else:
        WcwT = WcwT.astype(sdt).reshape(128, 1, 16, HID)
    bcw = _pack2(np.asarray(W_context_b, np.float32))
    WihT = _chunked(np.ascontiguousarray(np.asarray(gru_W_ih, np.float32).T), sdt)
    b_ih = np.asarray(gru_b_ih, np.float32)
    b_hh = np.asarray(gru_b_hh, np.float32)
    gates_bias = b_ih.copy()
    gates_bias[: 2 * HID] += b_hh[: 2 * HID]
    gates_bias = gates_bias.reshape(1, -1).astype(sdt)
    WhhT_f = _chunked(
        np.ascontiguousarray(np.asarray(gru_W_hh, np.float32).T), np.float32
    )  # [128, 2, 768]
    if SPLIT_HH:
        WhhT = np.ascontiguousarray(_split_hi_lo(WhhT_f))  # [128, 2(hi/lo), 2, 768]
    else:
        WhhT = np.ascontiguousarray(WhhT_f.astype(sdt).reshape(128, 1, 2, 3 * HID))
    bhn = _pack2(b_hh[2 * HID :]).astype(sdt)

    in_maps = []
    for c in range(NCORES):
        sl = slice(c * SH, (c + 1) * SH)
        H_c = np.zeros((SHP, NV), np.float32)
        H_c[:SH] = H[sl]
        X_c = np.zeros((SHP, EMB), np.float32)
        X_c[:SH] = X[sl]
        WoT_c = np.zeros((EMB, SHP), np.float32)
        WoT_c[:, :SH] = W_out_w[sl].T
        Wob_c = np.zeros((SHP,), np.float32)
        Wob_c[:SH] = W_out_b[sl]
        in_maps.append(
            {
                "h_mat": _chunked(H_c, fdt),
                "x_mat": _chunked(X_c, fdt),
                "wout_t": _chunked(WoT_c, odt),
                "wout_b": np.ascontiguousarray(Wob_c.reshape(KT, 128).T),
                "phe": phe,
                "wcw_t": WcwT,
                "wcw_b": bcw,
                "wih_t": WihT,
                "gates_bias": gates_bias,
                "whh_t": WhhT,
                "bhn": bhn,
            }
        )

    nc = _get_program()
    res = bass_utils.run_bass_kernel_spmd(
        nc,
        in_maps,
        core_ids=list(range(NCORES)),
        trace=bool(int(os.environ.get("KERNEL_TRACE", "0"))),
    )
    kernel.last_results = res
    out = np.concatenate(
        [np.asarray(res.results[c]["preds"], np.float32)[:SH] for c in range(NCORES)],
        axis=0,
    )
    return out


if __name__ == "__main__":
    np.random.seed(0)
    ins = {
        "phenotype_embs": np.random.randn(NTP, HID).astype(np.float32) * 0.02,
        "H": (np.random.rand(NC, NV) < 0.05).astype(np.float32),
        "X": np.random.randn(NC, EMB).astype(np.float32) * 0.02,
        "W_context_w": np.random.randn(HID, HID * NTP).astype(np.float32) * 0.02,
        "W_context_b": np.random.randn(HID).astype(np.float32) * 0.02,
        "gru_W_ih": np.random.randn(3 * HID, EMB).astype(np.float32) * 0.02,
        "gru_W_hh": np.random.randn(3 * HID, HID).astype(np.float32) * 0.02,
        "gru_b_ih": np.random.randn(3 * HID).astype(np.float32) * 0.02,
        "gru_b_hh": np.random.randn(3 * HID).astype(np.float32) * 0.02,
        "W_out_w": np.random.randn(NC, HID).astype(np.float32) * 0.02,
        "W_out_b": np.random.randn(NC).astype(np.float32) * 0.02,
        "num_visits": NV,
    }
    out = kernel(**ins)
    print("out", out.shape, out.dtype, out[:2, :4])


# revision 10
# speedup vs baseline: 2.6896x; 1.1482x over previous
"""Trainium2 Bass kernel for nn_Decoder (GRU decoder over phenotype codes).

Computation (see module docstring history / reference.py):
  h0 = W_cw @ phe_flat + b_cw                       (256,)
  G  = H.T @ X                                      (100, 256)   big GEMM, sharded over codes
  emb[t] = relu(G[t-1]), emb[0] = 0
  gates_x = emb @ W_ih.T + b_ih                     (100, 768)
  GRU scan over 100 steps (hidden 256)
  preds = sigmoid(hs @ W_out.T + W_out_b).T         (50000, 100)

Sharding: codes (50000) split over 8 cores (6250 -> padded 6272 each).
Front GEMM partial per core -> AllReduce(100x256) -> replicated GRU scan ->
sharded output GEMM + sigmoid.

All large tensors are pre-laid-out on the host into [128, ...] partition-major
form so each DMA is one contiguous descriptor per partition.
"""

import os
import sys

import numpy as np

sys.path.insert(0, "/opt/trn_rl_repo")

import ml_dtypes  # noqa: E402

import concourse.bass as bass  # noqa: E402
import concourse.mybir as mybir  # noqa: E402
import concourse.tile as tile  # noqa: E402
from concourse import bacc, bass_utils  # noqa: E402
from concourse.masks import make_identity  # noqa: E402

HID = 256
NTP = 8
EMB = 256
NC = 50000
NV = 100
NCORES = 8
SH = NC // NCORES  # 6250
KT = (SH + 127) // 128  # 49 k-tiles per shard
SHP = KT * 128  # 6272 padded codes per core

F32 = mybir.dt.float32
BF16 = mybir.dt.bfloat16

# ---- dtype knobs ----
DT_FRONT = BF16  # H, X, front GEMM
DT_SCAN = BF16  # gates GEMM, W_hh, h state, GX, identity
DT_OUT = BF16  # W_out, output GEMM
SPLIT_HH = False  # W_hh as bf16 hi+lo pair (fp32r-like accuracy, 2 passes)
SPLIT_CW = False  # same for the h0 GEMM weights
NPASS_HH = 2 if SPLIT_HH else 1
NPASS_CW = 2 if SPLIT_CW else 1


def _npdt(dt):
    return np.float32 if dt == F32 else ml_dtypes.bfloat16


def build_program():
    nc = bacc.Bacc(
        "TRN2",
        target_bir_lowering=False,
        debug=False,
        enable_asserts=False,
        num_devices=NCORES,
    )

    # ---- DRAM I/O (per core); all pre-laid-out [128, ...] on host ----
    H_d = nc.dram_tensor("h_mat", [128, KT, NV], DT_FRONT, kind="ExternalInput")
    X_d = nc.dram_tensor("x_mat", [128, KT, EMB], DT_FRONT, kind="ExternalInput")
    WoT_d = nc.dram_tensor("wout_t", [128, 2, SHP], DT_OUT, kind="ExternalInput")
    Wob_d = nc.dram_tensor("wout_b", [128, KT], F32, kind="ExternalInput")
    phe_d = nc.dram_tensor("phe", [128, 16], F32, kind="ExternalInput")
    WcwT_d = nc.dram_tensor(
        "wcw_t", [128, NPASS_CW, 16, HID], DT_SCAN, kind="ExternalInput"
    )
    bcw_d = nc.dram_tensor("wcw_b", [128, 2], F32, kind="ExternalInput")
    WihT_d = nc.dram_tensor("wih_t", [128, 2, 3 * HID], DT_SCAN, kind="ExternalInput")
    bih_d = nc.dram_tensor("gates_bias", [1, 3 * HID], DT_SCAN, kind="ExternalInput")
    WhhT_d = nc.dram_tensor(
        "whh_t", [128, NPASS_HH, 2, 3 * HID], DT_SCAN, kind="ExternalInput"
    )
    bhn_d = nc.dram_tensor("bhn", [128, 2], DT_SCAN, kind="ExternalInput")
    out_d = nc.dram_tensor("preds", [SHP, NV], F32, kind="ExternalOutput")

    with tile.TileContext(nc) as tc:
        with (
            tc.tile_pool(name="const", bufs=1) as cpool,
            tc.tile_pool(name="big", bufs=1) as bigpool,
            tc.tile_pool(name="work", bufs=2) as wpool,
            tc.tile_pool(name="dram", bufs=1, space="DRAM") as dpool,
        ):
            # ---- persistent SBUF tiles ----
            phe_sb = cpool.tile([128, 16], F32)
            WcwT_sb = cpool.tile([128, NPASS_CW, 16, HID], DT_SCAN)
            bcw_sb = cpool.tile([128, 2], F32)
            WihT_sb = cpool.tile([128, 2, 3 * HID], DT_SCAN)
            bih_sb = cpool.tile([1, 3 * HID], DT_SCAN)
            WhhT_sb = cpool.tile([128, NPASS_HH, 2, 3 * HID], DT_SCAN)
            bhn_sb = cpool.tile([128, 2], DT_SCAN)
            Wob_sb = cpool.tile([128, KT], F32)
            ID_sb = cpool.tile([128, 128], DT_SCAN)
            ones_sb = cpool.tile([1, NV], DT_SCAN)
            H_sb = bigpool.tile([128, KT, NV], DT_FRONT)
            X_sb = bigpool.tile([128, KT, EMB], DT_FRONT)
            WoT_sb = bigpool.tile([128, 2, SHP], DT_OUT)
            G_sb = cpool.tile([NV, EMB], F32)
            Gr_sb = cpool.tile([NV, EMB], DT_SCAN)
            embT_sb = cpool.tile([128, 2, NV], DT_SCAN)
            GX_sb = cpool.tile([128, NV, 4], DT_SCAN)
            GXN_sb = cpool.tile([128, NV, 2], DT_SCAN)
            hs_sb = cpool.tile([128, 2, NV], DT_SCAN)
            h0_sb = cpool.tile([128, 2], DT_SCAN)

            ar_in = dpool.tile([NV, EMB], F32)
            ar_out = dpool.tile([NV, EMB], F32)

            # ---- small DMAs + constants ----
            nc.sync.dma_start(phe_sb[:], phe_d[:])
            nc.sync.dma_start(WcwT_sb[:], WcwT_d[:])
            nc.sync.dma_start(bcw_sb[:], bcw_d[:])
            nc.sync.dma_start(WihT_sb[:], WihT_d[:])
            nc.sync.dma_start(bih_sb[:], bih_d[:])
            nc.sync.dma_start(WhhT_sb[:], WhhT_d[:])
            nc.sync.dma_start(bhn_sb[:], bhn_d[:])
            nc.sync.dma_start(Wob_sb[:], Wob_d[:])
            make_identity(nc, ID_sb[:])
            nc.gpsimd.memset(ones_sb[:], 1.0)
            nc.gpsimd.memset(embT_sb[:], 0.0)

            # ---- big DMAs (chunked so they spread across DMA engines) ----
            bounds = [0, 7, 14, 21, 28, 35, 42, KT]
            for i in range(len(bounds) - 1):
                ksl = slice(bounds[i], bounds[i + 1])
                eng = nc.sync if i % 2 == 0 else nc.gpsimd
                eng2 = nc.gpsimd if i % 2 == 0 else nc.sync
                eng.dma_start(H_sb[:, ksl, :], H_d[:, ksl, :])
                eng2.dma_start(X_sb[:, ksl, :], X_d[:, ksl, :])
            for kc in range(2):
                for i in range(8):
                    nsl = slice(i * (SHP // 8), (i + 1) * (SHP // 8))
                    eng = nc.sync if (i + kc) % 2 == 0 else nc.gpsimd
                    eng.dma_start(WoT_sb[:, kc, nsl], WoT_d[:, kc, nsl])

            # ---- h0 = W_cw @ phe + b_cw ----
            with tc.tile_pool(name="ps_h0", bufs=1, space="PSUM") as pp:
                ps_h0 = pp.tile([128, 2], F32)
                phe_cast = wpool.tile([128, 16], DT_SCAN, tag="phec")
                if DT_SCAN != F32:
                    nc.vector.tensor_copy(phe_cast[:], phe_sb[:])
                    phe_use = phe_cast
                else:
                    phe_use = phe_sb
                for m in range(2):
                    first = True
                    for p in range(NPASS_CW):
                        for k in range(16):
                            nc.tensor.matmul(
                                ps_h0[:, m : m + 1],
                                WcwT_sb[:, p, k, m * 128 : (m + 1) * 128],
                                phe_use[:, k : k + 1],
                                start=first,
                                stop=(p == NPASS_CW - 1 and k == 15),
                            )
                            first = False
                nc.vector.tensor_add(h0_sb[:], ps_h0[:], bcw_sb[:])

            # ---- front GEMM: G_partial = H_c.T @ X_c  (100 x 256) ----
            with tc.tile_pool(name="ps_g", bufs=1, space="PSUM") as pp:
                ps_g = pp.tile([NV, EMB], F32)
                for k in range(KT):
                    nc.tensor.matmul(
                        ps_g[:],
                        H_sb[:, k, :],
                        X_sb[:, k, :],
                        start=(k == 0),
                        stop=(k == KT - 1),
                    )
                Gp_sb = wpool.tile([NV, EMB], F32)
                nc.vector.tensor_copy(Gp_sb[:], ps_g[:])

            # ---- AllReduce the partial G ----
            nc.sync.dma_start(ar_in[:], Gp_sb[:])
            nc.gpsimd.collective_compute(
                "AllReduce",
                mybir.AluOpType.add,
                replica_groups=[list(range(NCORES))],
                ins=[ar_in.opt()],
                outs=[ar_out.opt()],
            )
            nc.sync.dma_start(G_sb[:], ar_out[:])

            # ---- emb.T = shifted relu(G).T  (256 x 100, col 0 = zeros) ----
            nc.vector.tensor_scalar(Gr_sb[:], G_sb[:], 0.0, None, mybir.AluOpType.max)
            with tc.tile_pool(name="ps_tr", bufs=2, space="PSUM") as pp:
                for c in range(2):
                    ps_tr = pp.tile([128, NV - 1], DT_SCAN, tag="tr")
                    nc.tensor.transpose(
                        ps_tr[:],
                        Gr_sb[0 : NV - 1, c * 128 : (c + 1) * 128],
                        ID_sb[0 : NV - 1, 0 : NV - 1],
                    )
                    nc.vector.tensor_copy(embT_sb[:, c, 1:NV], ps_tr[:])

            # ---- gates_x GEMM -> GX (rz, interleaved) and GXN (n) ----
            with tc.tile_pool(name="ps_gx", bufs=3, space="PSUM") as pp:
                for m in range(6):
                    ps_gx = pp.tile([128, NV], F32, tag="gx")
                    for kc in range(2):
                        nc.tensor.matmul(
                            ps_gx[:],
                            WihT_sb[:, kc, m * 128 : (m + 1) * 128],
                            embT_sb[:, kc, :],
                            start=(kc == 0),
                            stop=False,
                        )
                    nc.tensor.matmul(
                        ps_gx[:],
                        bih_sb[0:1, m * 128 : (m + 1) * 128],
                        ones_sb[0:1, :],
                        start=False,
                        stop=True,
                    )
                    if m < 4:
                        nc.vector.tensor_copy(GX_sb[:, :, m], ps_gx[:])
                    else:
                        nc.vector.tensor_copy(GXN_sb[:, :, m - 4], ps_gx[:])

            # ---- GRU scan ----
            with (
                tc.tile_pool(name="ps_scan", bufs=2, space="PSUM") as pp,
                tc.tile_pool(name="scan_t", bufs=3) as sp,
            ):
                for t in range(NV):
                    hcur = h0_sb if t == 0 else hs_sb[:, :, t - 1]
                    ps_rz = pp.tile([128, 4], F32, tag="scan_rz")
                    ps_n = pp.tile([128, 2], F32, tag="scan_n")
                    # seed psum with gate biases / rz inputs (sets has_written)
                    nc.tensor.matmul(
                        ps_rz[:], ID_sb[:], GX_sb[:, t, :], start=True, stop=False
                    )
                    nc.tensor.matmul(
                        ps_n[:], ID_sb[:], bhn_sb[:], start=True, stop=False
                    )
                    # rz passes first so sigmoid overlaps the n-gate passes
                    for m in range(4):
                        for p in range(NPASS_HH):
                            for kc in range(2):
                                nc.tensor.matmul(
                                    ps_rz[:, m : m + 1],
                                    WhhT_sb[:, p, kc, m * 128 : (m + 1) * 128],
                                    hcur[:, kc : kc + 1],
                                    start=False,
                                    stop=(m == 3 and p == NPASS_HH - 1 and kc == 1),
                                )
                    for m in range(4, 6):
                        for p in range(NPASS_HH):
                            for kc in range(2):
                                nc.tensor.matmul(
                                    ps_n[:, m - 4 : m - 3],
                                    WhhT_sb[:, p, kc, m * 128 : (m + 1) * 128],
                                    hcur[:, kc : kc + 1],
                                    start=False,
                                    stop=(m == 5 and p == NPASS_HH - 1 and kc == 1),
                                )
                    rz = sp.tile([128, 4], DT_SCAN, tag="rz")
                    nc.scalar.activation(
                        rz[:], ps_rz[:], mybir.ActivationFunctionType.Sigmoid
                    )
                    t1 = sp.tile([128, 2], DT_SCAN, tag="t1")
                    nc.vector.tensor_mul(t1[:], rz[:, 0:2], ps_n[:])
                    t2 = sp.tile([128, 2], DT_SCAN, tag="t2")
                    nc.vector.tensor_add(t2[:], t1[:], GXN_sb[:, t, :])
                    n_t = sp.tile([128, 2], DT_SCAN, tag="n")
                    nc.scalar.activation(
                        n_t[:], t2[:], mybir.ActivationFunctionType.Tanh
                    )
                    w_t = sp.tile([128, 2], DT_SCAN, tag="w")
                    nc.vector.tensor_scalar(
                        w_t[:],
                        rz[:, 2:4],
                        -1.0,
                        1.0,
                        mybir.AluOpType.mult,
                        mybir.AluOpType.add,
                    )
                    u_t = sp.tile([128, 2], DT_SCAN, tag="u")
                    nc.vector.tensor_mul(u_t[:], rz[:, 2:4], hcur)
                    v_t = sp.tile([128, 2], DT_SCAN, tag="v")
                    nc.vector.tensor_mul(v_t[:], n_t[:], w_t[:])
                    nc.vector.tensor_add(hs_sb[:, :, t], v_t[:], u_t[:])

            # ---- output GEMM + sigmoid + store ----
            with (
                tc.tile_pool(name="ps_out", bufs=6, space="PSUM") as pp,
                tc.tile_pool(name="pred", bufs=6) as op,
            ):
                for m in range(KT):
                    ps_o = pp.tile([128, NV], F32, tag="o")
                    for kc in range(2):
                        nc.tensor.matmul(
                            ps_o[:],
                            WoT_sb[:, kc, m * 128 : (m + 1) * 128],
                            hs_sb[:, kc, :],
                            start=(kc == 0),
                            stop=(kc == 1),
                        )
                    pred = op.tile([128, NV], F32, tag="p")
                    nc.scalar.activation(
                        pred[:],
                        ps_o[:],
                        mybir.ActivationFunctionType.Sigmoid,
                        bias=Wob_sb[:, m : m + 1],
                    )
                    eng = nc.sync if m % 2 == 0 else nc.gpsimd
                    eng.dma_start(out_d[m * 128 : (m + 1) * 128, :], pred[:])

    nc.compile()
    return nc


_PROG = None


def _get_program():
    global _PROG
    if _PROG is None:
        _PROG = build_program()
    return _PROG


def _pack2(v):
    # (256,) -> (128, 2) chunk-major: col c = v[128c:128c+128]
    return np.ascontiguousarray(v.reshape(2, 128).T)


def _chunked(a, dt):
    """(C*128, F...) row-major -> [128, C, F...] partition-major, contiguous."""
    c = a.shape[0] // 128
    out = np.ascontiguousarray(
        a.reshape((c, 128) + a.shape[1:]).swapaxes(0, 1).astype(dt)
    )
    return out


def _split_hi_lo(a):
    hi = a.astype(ml_dtypes.bfloat16)
    lo = (a - hi.astype(np.float32)).astype(ml_dtypes.bfloat16)
    return np.stack([hi, lo], axis=1)  # [128, 2, ...]


def kernel(
    phenotype_embs,
    H,
    X,
    W_context_w,
    W_context_b,
    gru_W_ih,
    gru_W_hh,
    gru_b_ih,
    gru_b_hh,
    W_out_w,
    W_out_b,
    num_visits,
):
    assert int(num_visits) == NV
    fdt = _npdt(DT_FRONT)
    sdt = _npdt(DT_SCAN)
    odt = _npdt(DT_OUT)

    H = np.asarray(H, np.float32)
    X = np.asarray(X, np.float32)
    W_out_w = np.asarray(W_out_w, np.float32)
    W_out_b = np.asarray(W_out_b, np.float32)

    # replicated small tensors
    phe = np.ascontiguousarray(
        np.asarray(phenotype_embs, np.float32).reshape(-1).reshape(16, 128).T
    )
    WcwT = _chunked(np.ascontiguousarray(np.asarray(W_context_w, np.float32).T), np.float32)
    # [128, 16, 256]
    if SPLIT_CW:
        WcwT = np.ascontiguousarray(
            _split_hi_lo(WcwT).reshape(128, 2, 16, HID)
        )
    else:
        WcwT = WcwT.astype(sdt).reshape(128, 1, 16, HID)
    bcw = _pack2(np.asarray(W_context_b, np.float32))
    WihT = _chunked(np.ascontiguousarray(np.asarray(gru_W_ih, np.float32).T), sdt)
    b_ih = np.asarray(gru_b_ih, np.float32)
    b_hh = np.asarray(gru_b_hh, np.float32)
    gates_bias = b_ih.copy()
    gates_bias[: 2 * HID] += b_hh[: 2 * HID]
    gates_bias = gates_bias.reshape(1, -1).astype(sdt)
    WhhT_f = _chunked(
        np.ascontiguousarray(np.asarray(gru_W_hh, np.float32).T), np.float32
    )  # [128, 2, 768]
    if SPLIT_HH:
        WhhT = np.ascontiguousarray(_split_hi_lo(WhhT_f))  # [128, 2(hi/lo), 2, 768]
    else:
        WhhT = np.ascontiguousarray(WhhT_f.astype(sdt).reshape(128, 1, 2, 3 * HID))
    bhn = _pack2(b_hh[2 * HID :]).astype(sdt)

    in_maps = []
    for c in range(NCORES):
        sl = slice(c * SH, (c + 1) * SH)
        H_c = np.zeros((SHP, NV), np.float32)
        H_c[:SH] = H[sl]
        X_c = np.zeros((SHP, EMB), np.float32)
        X_c[:SH] = X[sl]
        WoT_c = np.zeros((EMB, SHP), np.float32)
        WoT_c[:, :SH] = W_out_w[sl].T
        Wob_c = np.zeros((SHP,), np.float32)
        Wob_c[:SH] = W_out_b[sl]
        in_maps.append(
            {
                "h_mat": _chunked(H_c, fdt),
                "x_mat": _chunked(X_c, fdt),
                "wout_t": _chunked(WoT_c, odt),
                "wout_b": np.ascontiguousarray(Wob_c.reshape(KT, 128).T),
                "phe": phe,
                "wcw_t": WcwT,
                "wcw_b": bcw,
                "wih_t": WihT,
                "gates_bias": gates_bias,
                "whh_t": WhhT,
                "bhn": bhn,
            }
        )

    nc = _get_program()
    res = bass_utils.run_bass_kernel_spmd(
        nc,
        in_maps,
        core_ids=list(range(NCORES)),
        trace=bool(int(os.environ.get("KERNEL_TRACE", "0"))),
    )
    kernel.last_results = res
    out = np.concatenate(
        [np.asarray(res.results[c]["preds"], np.float32)[:SH] for c in range(NCORES)],
        axis=0,
    )
    return out


if __name__ == "__main__":
    np.random.seed(0)
    ins = {
        "phenotype_embs": np.random.randn(NTP, HID).astype(np.float32) * 0.02,
        "H": (np.random.rand(NC, NV) < 0.05).astype(np.float32),
        "X": np.random.randn(NC, EMB).astype(np.float32) * 0.02,
        "W_context_w": np.random.randn(HID, HID * NTP).astype(np.float32) * 0.02,
        "W_context_b": np.random.randn(HID).astype(np.float32) * 0.02,
        "gru_W_ih": np.random.randn(3 * HID, EMB).astype(np.float32) * 0.02,
        "gru_W_hh": np.random.randn(3 * HID, HID).astype(np.float32) * 0.02,
        "gru_b_ih": np.random.randn(3 * HID).astype(np.float32) * 0.02,
        "gru_b_hh": np.random.randn(3 * HID).astype(np.float32) * 0.02,
        "W_out_w": np.random.randn(NC, HID).astype(np.float32) * 0.02,
        "W_out_b": np.random.randn(NC).astype(np.float32) * 0.02,
        "num_visits": NV,
    }
    out = kernel(**ins)
    print("out", out.shape, out.dtype, out[:2, :4])
